# revision 6
# baseline (speedup 1.0000x reference)
"""Trainium2 Bass kernel for nn_DilatedResSkipBlock.

Reference math (per batch element b):
    w      = weight_norm(conv_v, conv_g)                  # [256, 128, 3]
    h      = causal_dilated_conv(x, w, dil=2, pad_left=4) + conv_b
    a, bb  = split(h, 2)                                  # [128, T] each
    c      = lc_w @ condition                             # [256, T]
    ca, cb = split(c, 2)
    g      = tanh(a + ca) * sigmoid(bb + cb)              # [128, T]
    s      = skip_w @ g + skip_b
    o      = out_w @ g + out_b + x
    return (o, s)

Sharding: data-parallel over batch -- 8 batch elements, one per NeuronCore.
Each core processes its full [128, 32768] time axis, so the dilated conv
needs no cross-core halo exchange.

Per-core kernel: time axis tiled at 2048 cols per DMA tile, 512 cols per
PSUM subtile.  x and condition stream in as bf16 (halves input HBM
traffic vs fp32); the conv/lc matmuls run in bf16 (1 PE cycle/row, same
as fp32r).  The skip/out 1x1 convs stay in float32r on an f32r copy of g
to hold the skip output's precision (its scale is 3x smaller than o's).

Engine split per 512-col subtile (PE is the bottleneck at 10 matmuls
= 2133 ns; everything else must stay under that):
    PE : 3 conv taps + 1 lc matmul per gate half (bf16, PSUM-accumulated),
         then skip/out matmuls (f32r)                       ~2133 ns
    ACT: ta = tanh(a+ba), tb = tanh(b/2+bb/2) [sigmoid-as-tanh],
         s  = copy(s_ps + skip_b)                           ~1700 ns
    DVE: g = ta*(1+tb) [f32r], o = o_ps + out_b + x         ~1300 ns
sigmoid(z) = (1 + tanh(z/2))/2: the b-half runs Tanh with scale=0.5 so
ACT only ever uses the Tanh table; the trailing 1/2 is folded into
halved skip/out weights, making g2 = ta*(1+tb) = 2*g.
Outputs are stored bf16 (halves output DMA) and upcast on the host.
"""

import numpy as np

RES, GATE, K, DIL, CIN = 128, 256, 3, 2, 80
PAD = (K - 1) * DIL  # 4
B, T = 8, 32768
N_CORES = 8
TILE = 2048   # columns per DMA tile
SUB = 512     # columns per PSUM subtile (one PSUM bank of fp32)
N_TILES = T // TILE
N_SUB = TILE // SUB

# wtsb packing layout ([128, 1024] bf16 dram input):
#   cols 0:768     conv lhsT, 6 blocks of 128: block (h*3+k) = w[h*128:(h+1)*128, :, k].T
#   cols 768:896   lc_a lhsT  (rows 0:80 valid)
#   cols 896:1024  lc_b lhsT  (rows 0:80 valid)
# wtsr ([128, 256] f32 dram input, DMA'd with f32r output dtype):
#   cols 0:128 skip lhsT / 2, cols 128:256 out lhsT / 2
# biasf ([128, 4] f32): col 0 conv_b[:128], col 1 conv_b[128:]/2,
#   col 2 skip_b, col 3 out_b
WB_COLS = 1024
WR_COLS = 256

_CACHE = {}


def _build_nc(reps=1, defer_so=True, probe_taps=K):
    """defer_so: issue the skip/out matmuls for subtile n-1 after subtile n's
    conv/lc matmuls, so the PE never waits on the ACT->DVE g chain.
    probe_taps: TIMING PROBE ONLY -- build with fewer conv taps (wrong math).
    """
    import contextlib

    import concourse.bacc as bacc
    import concourse.tile as tile
    from concourse import mybir

    f32 = mybir.dt.float32
    f32r = mybir.dt.float32r
    bf16 = mybir.dt.bfloat16
    Act = mybir.ActivationFunctionType
    Alu = mybir.AluOpType

    nc = bacc.Bacc("TRN2", target_bir_lowering=False, debug=False,
                   num_devices=N_CORES)

    x_d = nc.dram_tensor("x", [RES, T], bf16, kind="ExternalInput").ap()
    c_d = nc.dram_tensor("condition", [CIN, T], bf16, kind="ExternalInput").ap()
    wb_d = nc.dram_tensor("wtsb", [128, WB_COLS], bf16, kind="ExternalInput").ap()
    wr_d = nc.dram_tensor("wtsr", [128, WR_COLS], f32, kind="ExternalInput").ap()
    bias_d = nc.dram_tensor("biasf", [128, 4], f32, kind="ExternalInput").ap()
    z_d = nc.dram_tensor("zpad", [128, PAD], bf16, kind="ExternalInput").ap()
    o_d = nc.dram_tensor("o", [RES, T], bf16, kind="ExternalOutput").ap()
    s_d = nc.dram_tensor("s", [RES, T], bf16, kind="ExternalOutput").ap()

    with tile.TileContext(nc) as tc:
        with (
            tc.tile_pool(name="wpool", bufs=1) as wpool,
            tc.tile_pool(name="io", bufs=4) as io,
            tc.tile_pool(name="work", bufs=3) as work,
            tc.tile_pool(name="psum", bufs=2, space="PSUM") as psum,
        ):
            wb = wpool.tile([128, WB_COLS], bf16)
            nc.sync.dma_start(wb[:], wb_d[:])
            wr = wpool.tile([128, WR_COLS], f32r)
            nc.sync.dma_start(wr[:], wr_d[:].bitcast(f32r))
            bias = wpool.tile([128, 4], f32)
            nc.sync.dma_start(bias[:], bias_d[:])

            def conv_lhsT(h, k):
                c0 = (h * 3 + k) * 128
                return wb[:, c0:c0 + 128]

            lc_lhsT = [wb[0:CIN, 768:896], wb[0:CIN, 896:1024]]
            skip_lhsT = wr[:, 0:128]
            out_lhsT = wr[:, 128:256]
            bias_a = bias[:, 0:1]
            bias_b = bias[:, 1:2]
            skip_b = bias[:, 2:3]
            out_b = bias[:, 3:4]

            rep_loop = (tc.For_i(0, reps, 1) if reps > 1
                        else contextlib.nullcontext())
            with rep_loop:
                state = {"pending": None}
                tiles = {}

                def finish_pending(p):
                    # skip/out matmuls + bias/residual adds + tile DMA-out
                    # for the previous subtile
                    g, s_dst, o_dst, x_res, flush = p
                    s_ps = psum.tile([128, SUB], f32, tag="s")
                    o_ps = psum.tile([128, SUB], f32, tag="o")
                    nc.tensor.matmul(s_ps[:], skip_lhsT, g[:],
                                     start=True, stop=True)
                    nc.tensor.matmul(o_ps[:], out_lhsT, g[:],
                                     start=True, stop=True)
                    return (s_ps, o_ps, s_dst, o_dst, x_res, flush)

                def finish_post(q):
                    s_ps, o_ps, s_dst, o_dst, x_res, flush = q
                    nc.scalar.activation(s_dst, s_ps[:], Act.Identity,
                                         bias=skip_b)
                    nc.vector.scalar_tensor_tensor(
                        o_dst, o_ps[:], out_b, x_res,
                        op0=Alu.add, op1=Alu.add)
                    if flush is not None:
                        ti = flush
                        t0 = ti * TILE
                        nc.sync.dma_start(o_d[:, t0:t0 + TILE],
                                          tiles[ti][2][:])
                        nc.sync.dma_start(s_d[:, t0:t0 + TILE],
                                          tiles[ti][3][:])

                for n in range(N_TILES * N_SUB):
                    ti, sft = divmod(n, N_SUB)
                    if sft == 0:
                        t0 = ti * TILE
                        x_t = io.tile([RES, TILE + PAD], bf16, tag="x")
                        if ti == 0:
                            nc.sync.dma_start(x_t[:, 0:PAD], z_d[:])
                            nc.sync.dma_start(x_t[:, PAD:], x_d[:, 0:TILE])
                        else:
                            nc.sync.dma_start(x_t[:],
                                              x_d[:, t0 - PAD:t0 + TILE])
                        c_t = io.tile([CIN, TILE], bf16, tag="cond")
                        nc.sync.dma_start(c_t[:], c_d[:, t0:t0 + TILE])
                        o_t = io.tile([RES, TILE], bf16, tag="o")
                        s_t = io.tile([RES, TILE], bf16, tag="s")
                        tiles[ti] = (x_t, c_t, o_t, s_t)
                        tiles.pop(ti - 2, None)
                    x_t, c_t, o_t, s_t = tiles[ti]
                    lo = sft * SUB

                    a_ps = psum.tile([128, SUB], f32, tag="a")
                    b_ps = psum.tile([128, SUB], f32, tag="b")
                    for h, ps in ((0, a_ps), (1, b_ps)):
                        for k in range(probe_taps):
                            nc.tensor.matmul(
                                ps[:], conv_lhsT(h, k),
                                x_t[:, lo + DIL * k:lo + DIL * k + SUB],
                                start=(k == 0), stop=False)
                        nc.tensor.matmul(ps[:], lc_lhsT[h],
                                         c_t[:, lo:lo + SUB],
                                         start=False, stop=True)

                    queued = None
                    if defer_so and state["pending"] is not None:
                        queued = finish_pending(state["pending"])

                    ta = work.tile([128, SUB], f32, tag="ta")
                    tb = work.tile([128, SUB], f32, tag="tb")
                    nc.scalar.activation(ta[:], a_ps[:], Act.Tanh, bias=bias_a)
                    nc.scalar.activation(tb[:], b_ps[:], Act.Tanh,
                                         bias=bias_b, scale=0.5)
                    g = work.tile([128, SUB], f32r, tag="g")
                    nc.vector.scalar_tensor_tensor(
                        g[:], tb[:], 1.0, ta[:], op0=Alu.add, op1=Alu.mult)

                    if queued is not None:
                        finish_post(queued)

                    cur = (g, s_t[:, lo:lo + SUB], o_t[:, lo:lo + SUB],
                           x_t[:, PAD + lo:PAD + lo + SUB],
                           ti if sft == N_SUB - 1 else None)
                    if defer_so:
                        state["pending"] = cur
                    else:
                        finish_post(finish_pending(cur))

                if defer_so:
                    finish_post(finish_pending(state["pending"]))
                    state["pending"] = None

    nc.compile()
    return nc


def _get_nc(reps=1):
    key = ("nc", reps)
    if key not in _CACHE:
        _CACHE[key] = _build_nc(reps)
    return _CACHE[key]


def _pack_wts(conv_v, conv_g, conv_b, lc_v, lc_g, skip_v, skip_g, skip_b,
              out_v, out_g, out_b):
    import ml_dtypes

    def wn(v, g):
        norm = np.sqrt(np.sum(v * v, axis=(1, 2), keepdims=True))
        return v * (g.reshape(-1, 1, 1) / norm)

    conv_w = wn(conv_v, conv_g)            # [256, 128, 3]
    lc_w = wn(lc_v, lc_g)[:, :, 0]         # [256, 80]
    skip_w = wn(skip_v, skip_g)[:, :, 0]   # [128, 128]
    out_w = wn(out_v, out_g)[:, :, 0]      # [128, 128]

    wb = np.zeros((128, WB_COLS), np.float32)
    for h in range(2):
        for k in range(K):
            c0 = (h * 3 + k) * 128
            wb[:, c0:c0 + 128] = conv_w[h * 128:(h + 1) * 128, :, k].T
    wb[0:CIN, 768:896] = lc_w[0:128].T
    wb[0:CIN, 896:1024] = lc_w[128:256].T

    wr = np.zeros((128, WR_COLS), np.float32)
    wr[:, 0:128] = skip_w.T * 0.5
    wr[:, 128:256] = out_w.T * 0.5

    biasf = np.zeros((128, 4), np.float32)
    biasf[:, 0] = conv_b[0:128]
    biasf[:, 1] = conv_b[128:256] * 0.5
    biasf[:, 2] = skip_b
    biasf[:, 3] = out_b
    return wb.astype(ml_dtypes.bfloat16), wr, biasf


def _prepare_in_maps(inputs):
    """Host-side packing: full fp32 inputs -> per-core input dicts."""
    import ml_dtypes

    f = lambda k: np.ascontiguousarray(np.asarray(inputs[k], dtype=np.float32))
    x = f("x").astype(ml_dtypes.bfloat16)
    cond = f("condition").astype(ml_dtypes.bfloat16)
    wb, wr, biasf = _pack_wts(
        f("conv_v"), f("conv_g"), f("conv_b"), f("lc_v"), f("lc_g"),
        f("skip_v"), f("skip_g"), f("skip_b"), f("out_v"), f("out_g"),
        f("out_b"))
    zpad = np.zeros((128, PAD), ml_dtypes.bfloat16)
    return [{"x": x[b], "condition": cond[b], "wtsb": wb, "wtsr": wr,
             "biasf": biasf, "zpad": zpad} for b in range(N_CORES)]


def run(inputs, trace=False, **trace_kwargs):
    from concourse.bass_utils import run_bass_kernel_spmd

    in_maps = _prepare_in_maps(inputs)
    nc = _get_nc()
    res = run_bass_kernel_spmd(nc, in_maps, list(range(N_CORES)),
                               trace=trace, **trace_kwargs)
    o = np.stack([res.results[b]["o"] for b in range(N_CORES)]).astype(np.float32)
    s = np.stack([res.results[b]["s"] for b in range(N_CORES)]).astype(np.float32)
    return (o, s), res


def kernel(**inputs):
    out, _ = run(inputs, trace=False)
    return out


def _make_device_runner(nc):
    """jit-compiled 8-core runner with device-resident inputs (no donation,
    no per-call host transfer) for wall-clock timing."""
    import jax
    import numpy as np
    from jax.experimental.shard_map import shard_map
    from jax.sharding import Mesh, NamedSharding, PartitionSpec

    from concourse import mybir
    from concourse.bass2jax import (_bass_exec_p, install_neuronx_cc_hook,
                                    partition_id_tensor)

    install_neuronx_cc_hook()
    partition_name = (nc.partition_id_tensor.name
                      if nc.partition_id_tensor else None)
    in_names, out_names, out_avals, zero_outs = [], [], [], []
    for alloc in nc.m.functions[0].allocations:
        if not isinstance(alloc, mybir.MemoryLocationSet):
            continue
        name = alloc.memorylocations[0].name
        if alloc.kind == "ExternalInput":
            if name != partition_name:
                in_names.append(name)
        elif alloc.kind == "ExternalOutput":
            shape = tuple(alloc.tensor_shape)
            dtype = mybir.dt.np(alloc.dtype)
            out_names.append(name)
            out_avals.append(jax.core.ShapedArray(shape, dtype))
            zero_outs.append(np.zeros(shape, dtype))
    n_params = len(in_names)
    all_in_names = list(in_names) + list(out_names)
    if partition_name is not None:
        all_in_names.append(partition_name)

    def _body(*args):
        operands = list(args)
        if partition_name is not None:
            operands.append(partition_id_tensor())
        return tuple(_bass_exec_p.bind(
            *operands,
            out_avals=tuple(out_avals),
            in_names=tuple(all_in_names),
            out_names=tuple(out_names),
            lowering_input_output_aliases=(),
            sim_require_finite=True,
            sim_require_nnan=True,
            nc=nc,
        ))

    devices = jax.devices()[:N_CORES]
    mesh = Mesh(np.asarray(devices), ("core",))
    spec = PartitionSpec("core")
    f = jax.jit(shard_map(_body, mesh=mesh,
                          in_specs=(spec,) * (n_params + len(out_names)),
                          out_specs=(spec,) * len(out_names),
                          check_rep=False),
                keep_unused=True)

    def put(per_core_arrays):
        # per_core_arrays: list over inputs of list over cores
        sharding = NamedSharding(mesh, spec)
        out = []
        for arrs in per_core_arrays:
            out.append(jax.device_put(
                np.concatenate(arrs, axis=0), sharding))
        return out

    return f, put, in_names, n_params, zero_outs


def measure_exec_ns(inputs, reps=512, iters=10):
    """Estimate per-invocation HW time via interleaved timing of reps=512 and
    reps=1024 kernels: ns = (wall[1024] - wall[512]) / 512.  Interleaving the
    two variants decorrelates slow drift in dispatch/axon overhead, and both
    runs are long enough that per-call overhead is a tiny fraction."""
    import statistics
    import time

    import jax

    in_maps = _prepare_in_maps(inputs)
    r_lo, r_hi = reps, reps * 2

    def prep(nc):
        fjit, put, in_names, n_params, zero_outs = _make_device_runner(nc)
        per_core = [[in_maps[b][n] for b in range(N_CORES)] for n in in_names]
        per_core += [[z for _ in range(N_CORES)] for z in zero_outs]
        dev_args = put(per_core)
        jax.block_until_ready(fjit(*dev_args))  # compile + warm
        return fjit, dev_args

    f_lo, a_lo = prep(_get_nc(r_lo))
    f_hi, a_hi = prep(_get_nc(r_hi))

    t_lo, t_hi = [], []
    for _ in range(iters):
        t0 = time.perf_counter()
        jax.block_until_ready(f_lo(*a_lo))
        t_lo.append(time.perf_counter() - t0)
        t0 = time.perf_counter()
        jax.block_until_ready(f_hi(*a_hi))
        t_hi.append(time.perf_counter() - t0)
    fmt = lambda ts: "[" + " ".join(f"{t * 1e3:.1f}" for t in ts) + "] ms"
    print(f"  wall[{r_lo}]  {fmt(t_lo)}")
    print(f"  wall[{r_hi}] {fmt(t_hi)}")
    deltas = sorted((h - l) / (r_hi - r_lo) * 1e9
                    for h, l in zip(t_hi, t_lo))
    med = statistics.median(deltas)
    nsmin = (min(t_hi) - min(t_lo)) / (r_hi - r_lo) * 1e9
    print(f"  paired deltas (ns/iter): "
          + " ".join(f"{d:.0f}" for d in deltas))
    print(f"  median delta {med:.0f} ns/iter, min delta {nsmin:.0f} ns/iter")
    return med
